# revision 11
# baseline (speedup 1.0000x reference)
"""GIN 2-layer (GINConv + MLP + BN + ReLU) x2 on 8 Trainium2 NeuronCores.

Sharding: dst-shard the 100k nodes into 8 contiguous ranges of 12500
(edges partitioned by dst core). Within a core, nodes are permuted by
descending in-degree into 98 blocks of 128 slots (44 trailing pads).
Aggregation = per-block gather-accumulate "rounds": round r gathers the
r-th in-edge's source row for each of the block's 128 nodes via
indirect DMA with CCE-add into the SBUF accumulator (OOB sentinel
indices accumulate zero). The MLP runs in feature-on-partition
(transposed) layout so biases/BN are per-partition and BN stats are
free-axis reductions. BatchNorm needs global batch stats, so the work
is split into 3 device launches:
  L1: aggregate + MLP1 -> h1_pre shards (node-major) + per-core BN sums
  (host concatenates shards / adds partial sums -- data movement only)
  L2: BN1+ReLU over the full table, aggregate + MLP2 -> h2_pre shards + sums
  L3: BN2+ReLU -> output shards
"""
import sys

sys.path.insert(0, "/opt/trn_rl_repo")

import numpy as np
from concourse import bass, mybir
import concourse.tile as tile
from concourse.bass_utils import run_bass_kernel_spmd
from concourse.masks import make_identity

N_NODES = 100000
N_CORES = 8
P = 128
PER_CORE = N_NODES // N_CORES          # 12500
BLOCKS = (PER_CORE + P - 1) // P       # 98
SLOTS = BLOCKS * P                     # 12544
TABLE_ROWS = N_CORES * SLOTS           # 100352
LAST_VALID = PER_CORE - (BLOCKS - 1) * P   # 84 valid cols in last block
F_IN, H1, H2 = 256, 128, 32
BN_EPS = 1e-5
F32 = mybir.dt.float32
I32 = mybir.dt.int32


# ---------------------------------------------------------------- wait split
def _split_sync_waits(nc, max_waits=1):
    """This container's walrus rejects >1 sync wait per instruction; hoist
    extras onto preceding same-engine NoOps."""
    ctr = 0
    for f in nc.m.functions:
        for bb in f.blocks:
            out, changed = [], False
            for inst in list(bb.instructions):
                si = getattr(inst, "sync_info", None)
                if si is not None and si.on_wait and len(si.on_wait) > max_waits:
                    waits = list(si.on_wait)
                    extra, keep = waits[:-max_waits], waits[-max_waits:]
                    for i in range(0, len(extra), max_waits):
                        ctr += 1
                        out.append(mybir.InstNoOp(
                            name=f"waitsplit-nop-{ctr}", ins=[], outs=[],
                            engine=inst.engine,
                            sync_info=mybir.SyncInfo(
                                on_wait=extra[i:i + max_waits], on_update=[]),
                        ))
                    inst.sync_info = mybir.SyncInfo(
                        on_wait=keep, on_update=list(si.on_update or []))
                    changed = True
                out.append(inst)
            if changed:
                bb.instructions = out
    return ctr


# ---------------------------------------------------------------- host plan
def _build_plan(edge_index):
    src = np.asarray(edge_index[0], dtype=np.int64)
    dst = np.asarray(edge_index[1], dtype=np.int64)
    core_of = dst // PER_CORE

    per_core, all_R = [], np.zeros((N_CORES, BLOCKS), dtype=np.int64)
    for c in range(N_CORES):
        sel = np.nonzero(core_of == c)[0]
        s_c = src[sel]
        d_c = dst[sel] - c * PER_CORE
        deg = np.bincount(d_c, minlength=PER_CORE)
        perm = np.argsort(-deg, kind="stable")
        pos_of = np.empty(PER_CORE, dtype=np.int64)
        pos_of[perm] = np.arange(PER_CORE)
        slot = pos_of[d_c]
        order = np.argsort(slot, kind="stable")
        s_c, slot = s_c[order], slot[order]
        first = np.searchsorted(slot, np.arange(SLOTS), side="left")
        counts = np.bincount(slot, minlength=SLOTS)
        rank = np.arange(len(slot)) - first[slot]
        blk = slot // P
        all_R[c] = counts.reshape(BLOCKS, P).max(axis=1)
        per_core.append(dict(perm=perm, pos_of=pos_of, s=s_c, slot=slot,
                             rank=rank, blk=blk))

    R = all_R.max(axis=0)
    offs = np.zeros(BLOCKS + 1, dtype=np.int64)
    offs[1:] = np.cumsum(R)
    TOT = int(offs[-1])

    tab_pos = np.empty(N_NODES, dtype=np.int64)
    for c in range(N_CORES):
        ids = np.arange(c * PER_CORE, (c + 1) * PER_CORE)
        tab_pos[ids] = c * SLOTS + per_core[c]["pos_of"]

    # combined idx tables: per block b the columns are
    # [self][round 0]...[round R[b]-1]; block b starts at b + offs[b]
    TOTC = BLOCKS + TOT
    starts = (np.arange(BLOCKS) + offs[:-1]).astype(np.int64)
    cores = []
    for c in range(N_CORES):
        pc = per_core[c]
        col = starts[pc["blk"]] + 1 + pc["rank"]
        row = pc["slot"] % P

        idx1 = np.full((P, TOTC), N_NODES, dtype=np.int32)
        idx1[row, col] = pc["s"]
        sl = np.arange(SLOTS)
        gl = np.where(sl < PER_CORE,
                      np.concatenate([pc["perm"] + c * PER_CORE,
                                      np.zeros(SLOTS - PER_CORE, np.int64)])[
                          np.minimum(sl, SLOTS - 1)],
                      0)
        # self column for block b at starts[b]
        idx1[:, starts] = gl.reshape(BLOCKS, P).T.astype(np.int32)

        idx2 = np.full((P, TOTC), TABLE_ROWS, dtype=np.int32)
        idx2[row, col] = tab_pos[pc["s"]]
        self2 = np.where(sl < PER_CORE, c * SLOTS + sl, 0)
        idx2[:, starts] = self2.reshape(BLOCKS, P).T.astype(np.int32)

        cores.append(dict(idx1=idx1, idx2=idx2, perm=pc["perm"]))

    return dict(R=R, offs=offs, TOT=TOT, TOTC=TOTC, starts=starts, cores=cores,
                tab_pos=tab_pos)


# ---------------------------------------------------------------- launches
def _gather_rounds(nc, tc, pools, tab_ap, idx_dram, b, start, nrounds, width):
    """Self (bypass) + nrounds accumulate gathers into a fresh acc tile.
    All indices are valid rows; pad slots point at a trailing zero row of
    the table (adds 0). No OOB/bounds path -- it wedges the device at
    scale in this toolchain."""
    idxp, accp = pools
    idx_t = idxp.tile([P, 1 + nrounds], I32)
    nc.sync.dma_start(out=idx_t[:], in_=idx_dram[:, start:start + 1 + nrounds])
    acc = accp.tile([P, width], F32)
    nc.gpsimd.indirect_dma_start(
        out=acc[:], out_offset=None, in_=tab_ap,
        in_offset=bass.IndirectOffsetOnAxis(ap=idx_t[:, 0:1], axis=0),
        compute_op=mybir.AluOpType.bypass)
    for r in range(nrounds):
        nc.gpsimd.indirect_dma_start(
            out=acc[:], out_offset=None, in_=tab_ap,
            in_offset=bass.IndirectOffsetOnAxis(ap=idx_t[:, 1 + r:2 + r], axis=0),
            compute_op=mybir.AluOpType.add)
    return acc


def _build_launch1(plan):
    nc = bass.Bass("TRN2", target_bir_lowering=False, debug=False,
                   num_devices=N_CORES)
    x = nc.dram_tensor("x", [N_NODES + 1, F_IN], F32, kind="ExternalInput")
    idx1 = nc.dram_tensor("idx1", [P, plan["TOTC"]], I32, kind="ExternalInput")
    w1a = nc.dram_tensor("w1a", [F_IN, H1], F32, kind="ExternalInput")
    b1a = nc.dram_tensor("b1a", [H1, 1], F32, kind="ExternalInput")
    w1b = nc.dram_tensor("w1b", [H1, H1], F32, kind="ExternalInput")
    b1b = nc.dram_tensor("b1b", [H1, 1], F32, kind="ExternalInput")
    h1o = nc.dram_tensor("h1o", [SLOTS, H1], F32, kind="ExternalOutput")
    stats = nc.dram_tensor("stats", [H1, 2], F32, kind="ExternalOutput")

    offs, R, starts = plan["offs"], plan["R"], plan["starts"]
    with tile.TileContext(nc) as tc:
        with (
            tc.tile_pool(name="const", bufs=1) as constp,
            tc.tile_pool(name="idxp", bufs=4) as idxp,
            tc.tile_pool(name="accp", bufs=8) as accp,
            tc.tile_pool(name="work", bufs=4) as workp,
            tc.tile_pool(name="psum", bufs=1, space="PSUM") as psump,
        ):
            ident = constp.tile([P, P], F32)
            make_identity(nc, ident[:])
            w1a_t = constp.tile([P, 2 * H1], F32)   # [128, 256]: two K-chunks
            nc.sync.dma_start(out=w1a_t[:, 0:H1], in_=w1a[0:P, :])
            nc.sync.dma_start(out=w1a_t[:, H1:2 * H1], in_=w1a[P:2 * P, :])
            w1b_t = constp.tile([H1, H1], F32)
            nc.sync.dma_start(out=w1b_t[:], in_=w1b[:, :])
            b1a_t = constp.tile([H1, 1], F32)
            nc.sync.dma_start(out=b1a_t[:], in_=b1a[:, :])
            b1b_t = constp.tile([H1, 1], F32)
            nc.sync.dma_start(out=b1b_t[:], in_=b1b[:, :])
            mask = constp.tile([P, P], F32)
            nc.vector.memset(mask[:], 1.0)
            nc.vector.memset(mask[:, LAST_VALID:], 0.0)
            s_sum = constp.tile([H1, 1], F32)
            s_sq = constp.tile([H1, 1], F32)
            nc.vector.memset(s_sum[:], 0.0)
            nc.vector.memset(s_sq[:], 0.0)

            for b in range(BLOCKS):
                acc = _gather_rounds(nc, tc, (idxp, accp), x[:, :], idx1,
                                     b, int(starts[b]), int(R[b]), F_IN)
                # transpose acc -> accT (2 x [128,128])
                accT = workp.tile([P, F_IN], F32, tag="accT")
                for cchunk in range(2):
                    pT = psump.tile([P, P], F32, tag="pT", space="PSUM")
                    nc.tensor.transpose(
                        out=pT[:], in_=acc[:, cchunk * P:(cchunk + 1) * P],
                        identity=ident[:])
                    nc.vector.tensor_copy(
                        out=accT[:, cchunk * P:(cchunk + 1) * P], in_=pT[:])
                ps1 = psump.tile([H1, P], F32, tag="ps1", space="PSUM")
                nc.tensor.matmul(out=ps1[:], lhsT=w1a_t[:, 0:H1],
                                 rhs=accT[:, 0:P], start=True, stop=False)
                nc.tensor.matmul(out=ps1[:], lhsT=w1a_t[:, H1:2 * H1],
                                 rhs=accT[:, P:2 * P], start=False, stop=True)
                r1 = workp.tile([H1, P], F32, tag="r1")
                nc.scalar.activation(r1[:], ps1[:],
                                     mybir.ActivationFunctionType.Relu,
                                     bias=b1a_t[:, 0:1])
                ps2 = psump.tile([H1, P], F32, tag="ps2", space="PSUM")
                nc.tensor.matmul(out=ps2[:], lhsT=w1b_t[:], rhs=r1[:],
                                 start=True, stop=True)
                hpre = workp.tile([H1, P], F32, tag="hpre")
                nc.scalar.activation(hpre[:], ps2[:],
                                     mybir.ActivationFunctionType.Identity,
                                     bias=b1b_t[:, 0:1])
                # stats (mask the pad columns of the last block)
                if b == BLOCKS - 1:
                    hstat = workp.tile([H1, P], F32, tag="hstat")
                    nc.vector.tensor_tensor(out=hstat[:], in0=hpre[:],
                                            in1=mask[0:H1, :],
                                            op=mybir.AluOpType.mult)
                else:
                    hstat = hpre
                part = workp.tile([H1, 1], F32, tag="part")
                nc.vector.tensor_reduce(out=part[:], in_=hstat[:],
                                        axis=mybir.AxisListType.X,
                                        op=mybir.AluOpType.add)
                nc.vector.tensor_add(out=s_sum[:], in0=s_sum[:], in1=part[:])
                sq = workp.tile([H1, P], F32, tag="sq")
                nc.scalar.square(sq[:], hstat[:])
                part2 = workp.tile([H1, 1], F32, tag="part2")
                nc.vector.tensor_reduce(out=part2[:], in_=sq[:],
                                        axis=mybir.AxisListType.X,
                                        op=mybir.AluOpType.add)
                nc.vector.tensor_add(out=s_sq[:], in0=s_sq[:], in1=part2[:])
                # store node-major
                psT = psump.tile([P, H1], F32, tag="psT", space="PSUM")
                nc.tensor.transpose(out=psT[:], in_=hpre[:],
                                    identity=ident[0:H1, 0:H1])
                hrow = workp.tile([P, H1], F32, tag="hrow")
                nc.vector.tensor_copy(out=hrow[:], in_=psT[:])
                nc.sync.dma_start(out=h1o[b * P:(b + 1) * P, :], in_=hrow[:])

            nc.sync.dma_start(out=stats[:, 0:1], in_=s_sum[:])
            nc.sync.dma_start(out=stats[:, 1:2], in_=s_sq[:])

    return nc


def _emit_scale_shift(nc, constp, psump, statsc, g, be, nf, n_count):
    """Device BN coefficient computation from raw sums.
    statsc [nf,2] (sum, sumsq); returns full [P, nf] scale/shift tiles."""
    mean = constp.tile([nf, 1], F32)
    nc.scalar.mul(mean[:], statsc[:, 0:1], 1.0 / n_count)
    ex2 = constp.tile([nf, 1], F32)
    nc.scalar.mul(ex2[:], statsc[:, 1:2], 1.0 / n_count)
    msq = constp.tile([nf, 1], F32)
    nc.vector.tensor_tensor(out=msq[:], in0=mean[:], in1=mean[:],
                            op=mybir.AluOpType.mult)
    var = constp.tile([nf, 1], F32)
    nc.vector.tensor_tensor(out=var[:], in0=ex2[:], in1=msq[:],
                            op=mybir.AluOpType.subtract)
    veps = constp.tile([nf, 1], F32)
    nc.vector.tensor_scalar_add(veps[:], var[:], BN_EPS)
    std = constp.tile([nf, 1], F32)
    nc.scalar.sqrt(std[:], veps[:])
    istd = constp.tile([nf, 1], F32)
    nc.vector.reciprocal(istd[:], std[:])
    scale_c = constp.tile([nf, 1], F32)
    nc.vector.tensor_tensor(out=scale_c[:], in0=g[:], in1=istd[:],
                            op=mybir.AluOpType.mult)
    mscale = constp.tile([nf, 1], F32)
    nc.vector.tensor_tensor(out=mscale[:], in0=mean[:], in1=scale_c[:],
                            op=mybir.AluOpType.mult)
    shift_c = constp.tile([nf, 1], F32)
    nc.vector.tensor_tensor(out=shift_c[:], in0=be[:], in1=mscale[:],
                            op=mybir.AluOpType.subtract)
    # broadcast to [P, nf] via rank-1 matmul: ones_row^T (x) coef_row
    ident1 = constp.tile([nf, nf], F32)
    make_identity(nc, ident1[:])
    ones_row = constp.tile([1, P], F32)
    nc.vector.memset(ones_row[:], 1.0)
    out_tiles = []
    for i, coef in enumerate((scale_c, shift_c)):
        prow_ps = psump.tile([1, nf], F32, tag=f"prow{i}", space="PSUM")
        nc.tensor.transpose(out=prow_ps[:], in_=coef[:],
                            identity=ident1[0:nf, 0:nf])
        crow = constp.tile([1, nf], F32, tag=f"crow{i}")
        nc.vector.tensor_copy(out=crow[:], in_=prow_ps[:])
        full_ps = psump.tile([P, nf], F32, tag=f"fullp{i}", space="PSUM")
        nc.tensor.matmul(out=full_ps[:], lhsT=ones_row[:], rhs=crow[:],
                         start=True, stop=True)
        full = constp.tile([P, nf], F32, tag=f"coef_full{i}")
        nc.vector.tensor_copy(out=full[:], in_=full_ps[:])
        out_tiles.append(full)
    return out_tiles


def _build_launch2(plan):
    nc = bass.Bass("TRN2", target_bir_lowering=False, debug=False,
                   num_devices=N_CORES)
    tab1 = nc.dram_tensor("tab1", [TABLE_ROWS, H1], F32, kind="ExternalInput")
    stats1 = nc.dram_tensor("stats1", [H1, 2], F32, kind="ExternalInput")
    g1 = nc.dram_tensor("g1", [H1, 1], F32, kind="ExternalInput")
    be1 = nc.dram_tensor("be1", [H1, 1], F32, kind="ExternalInput")
    idx2 = nc.dram_tensor("idx2", [P, plan["TOTC"]], I32, kind="ExternalInput")
    w2a = nc.dram_tensor("w2a", [H1, H2], F32, kind="ExternalInput")
    b2a = nc.dram_tensor("b2a", [H2, 1], F32, kind="ExternalInput")
    w2b = nc.dram_tensor("w2b", [H2, H2], F32, kind="ExternalInput")
    b2b = nc.dram_tensor("b2b", [H2, 1], F32, kind="ExternalInput")
    h2o = nc.dram_tensor("h2o", [SLOTS, H2], F32, kind="ExternalOutput")
    stats = nc.dram_tensor("stats", [H2, 2], F32, kind="ExternalOutput")
    relu_tab = nc.dram_tensor("relu_tab", [TABLE_ROWS + 1, H1], F32,
                              kind="Internal")

    offs, R, starts = plan["offs"], plan["R"], plan["starts"]
    # Phase 1 (own TileContext => hard barrier before phase 2): BN1 coeffs
    # + normalize/relu the full table into relu_tab.
    with tile.TileContext(nc) as tc:
        with (
            tc.tile_pool(name="const1", bufs=1) as constp,
            tc.tile_pool(name="norm", bufs=4) as normp,
            tc.tile_pool(name="psum1", bufs=1, space="PSUM") as psump,
        ):
            stats1_t = constp.tile([H1, 2], F32)
            nc.sync.dma_start(out=stats1_t[:], in_=stats1[:, :])
            g1_t = constp.tile([H1, 1], F32)
            nc.sync.dma_start(out=g1_t[:], in_=g1[:, :])
            be1_t = constp.tile([H1, 1], F32)
            nc.sync.dma_start(out=be1_t[:], in_=be1[:, :])
            scale_full, shift_full = _emit_scale_shift(
                nc, constp, psump, stats1_t, g1_t, be1_t, H1, N_NODES)

            # normalize + relu the full table: 98 strided tiles of 1024 rows
            CH = 8  # rows per partition per tile
            tabv = tab1[:, :].rearrange("(t p j) f -> t p (j f)", p=P, j=CH)
            reluv = relu_tab[0:TABLE_ROWS, :].rearrange("(t p j) f -> t p (j f)", p=P, j=CH)
            n_t = TABLE_ROWS // (P * CH)
            sc_b = scale_full[:].rearrange("p f -> p () f").to_broadcast(
                [P, CH, H1])
            sh_b = shift_full[:].rearrange("p f -> p () f").to_broadcast(
                [P, CH, H1])
            for t in range(n_t):
                nt = normp.tile([P, CH * H1], F32, tag="nt")
                nc.sync.dma_start(out=nt[:], in_=tabv[t])
                sc = normp.tile([P, CH * H1], F32, tag="sc")
                nc.vector.tensor_tensor(
                    out=sc[:].rearrange("p (j f) -> p j f", f=H1),
                    in0=nt[:].rearrange("p (j f) -> p j f", f=H1),
                    in1=sc_b,
                    op=mybir.AluOpType.mult)
                nc.vector.tensor_tensor(
                    out=sc[:].rearrange("p (j f) -> p j f", f=H1),
                    in0=sc[:].rearrange("p (j f) -> p j f", f=H1),
                    in1=sh_b,
                    op=mybir.AluOpType.add)
                nc.scalar.activation(sc[:], sc[:],
                                     mybir.ActivationFunctionType.Relu)
                nc.sync.dma_start(out=reluv[t], in_=sc[:])
            zrow = normp.tile([1, H1], F32, tag="zrow")
            nc.vector.memset(zrow[:], 0.0)
            nc.sync.dma_start(out=relu_tab[TABLE_ROWS:TABLE_ROWS + 1, :],
                              in_=zrow[:])

    # Phase 2: aggregate from relu_tab + MLP2 + stats.
    with tile.TileContext(nc) as tc:
        with (
            tc.tile_pool(name="const", bufs=1) as constp,
            tc.tile_pool(name="idxp", bufs=4) as idxp,
            tc.tile_pool(name="accp", bufs=8) as accp,
            tc.tile_pool(name="work", bufs=4) as workp,
            tc.tile_pool(name="psum", bufs=1, space="PSUM") as psump,
        ):
            ident = constp.tile([P, P], F32)
            make_identity(nc, ident[:])
            # layer-2 weights/consts
            w2a_t = constp.tile([H1, H2], F32)
            nc.sync.dma_start(out=w2a_t[:], in_=w2a[:, :])
            w2b_t = constp.tile([H2, H2], F32)
            nc.sync.dma_start(out=w2b_t[:], in_=w2b[:, :])
            b2a_t = constp.tile([H2, 1], F32)
            nc.sync.dma_start(out=b2a_t[:], in_=b2a[:, :])
            b2b_t = constp.tile([H2, 1], F32)
            nc.sync.dma_start(out=b2b_t[:], in_=b2b[:, :])
            mask = constp.tile([H2, P], F32)
            nc.vector.memset(mask[:], 1.0)
            nc.vector.memset(mask[:, LAST_VALID:], 0.0)
            s_sum = constp.tile([H2, 1], F32)
            s_sq = constp.tile([H2, 1], F32)
            nc.vector.memset(s_sum[:], 0.0)
            nc.vector.memset(s_sq[:], 0.0)

            for b in range(BLOCKS):
                acc = _gather_rounds(nc, tc, (idxp, accp), relu_tab[:, :],
                                     idx2, b, int(starts[b]),
                                     int(R[b]), H1)
                accT = workp.tile([P, P], F32, tag="accT")
                pT = psump.tile([P, P], F32, tag="pT", space="PSUM")
                nc.tensor.transpose(out=pT[:], in_=acc[:], identity=ident[:])
                nc.vector.tensor_copy(out=accT[:], in_=pT[:])
                ps1 = psump.tile([H2, P], F32, tag="ps1", space="PSUM")
                nc.tensor.matmul(out=ps1[:], lhsT=w2a_t[:], rhs=accT[:],
                                 start=True, stop=True)
                r1 = workp.tile([H2, P], F32, tag="r1")
                nc.scalar.activation(r1[:], ps1[:],
                                     mybir.ActivationFunctionType.Relu,
                                     bias=b2a_t[:, 0:1])
                ps2 = psump.tile([H2, P], F32, tag="ps2", space="PSUM")
                nc.tensor.matmul(out=ps2[:], lhsT=w2b_t[:], rhs=r1[:],
                                 start=True, stop=True)
                hpre = workp.tile([H2, P], F32, tag="hpre")
                nc.scalar.activation(hpre[:], ps2[:],
                                     mybir.ActivationFunctionType.Identity,
                                     bias=b2b_t[:, 0:1])
                if b == BLOCKS - 1:
                    hstat = workp.tile([H2, P], F32, tag="hstat")
                    nc.vector.tensor_tensor(out=hstat[:], in0=hpre[:],
                                            in1=mask[:],
                                            op=mybir.AluOpType.mult)
                else:
                    hstat = hpre
                part = workp.tile([H2, 1], F32, tag="part")
                nc.vector.tensor_reduce(out=part[:], in_=hstat[:],
                                        axis=mybir.AxisListType.X,
                                        op=mybir.AluOpType.add)
                nc.vector.tensor_add(out=s_sum[:], in0=s_sum[:], in1=part[:])
                sq = workp.tile([H2, P], F32, tag="sq")
                nc.scalar.square(sq[:], hstat[:])
                part2 = workp.tile([H2, 1], F32, tag="part2")
                nc.vector.tensor_reduce(out=part2[:], in_=sq[:],
                                        axis=mybir.AxisListType.X,
                                        op=mybir.AluOpType.add)
                nc.vector.tensor_add(out=s_sq[:], in0=s_sq[:], in1=part2[:])
                psT = psump.tile([P, H2], F32, tag="psT", space="PSUM")
                nc.tensor.transpose(out=psT[:], in_=hpre[:],
                                    identity=ident[0:H2, 0:H2])
                hrow = workp.tile([P, H2], F32, tag="hrow")
                nc.vector.tensor_copy(out=hrow[:], in_=psT[:])
                nc.sync.dma_start(out=h2o[b * P:(b + 1) * P, :], in_=hrow[:])

            nc.sync.dma_start(out=stats[:, 0:1], in_=s_sum[:])
            nc.sync.dma_start(out=stats[:, 1:2], in_=s_sq[:])

    return nc


def _build_launch3():
    nc = bass.Bass("TRN2", target_bir_lowering=False, debug=False,
                   num_devices=N_CORES)
    h2i = nc.dram_tensor("h2i", [SLOTS, H2], F32, kind="ExternalInput")
    stats2 = nc.dram_tensor("stats2", [H2, 2], F32, kind="ExternalInput")
    g2 = nc.dram_tensor("g2", [H2, 1], F32, kind="ExternalInput")
    be2 = nc.dram_tensor("be2", [H2, 1], F32, kind="ExternalInput")
    outo = nc.dram_tensor("outo", [SLOTS, H2], F32, kind="ExternalOutput")

    with tile.TileContext(nc) as tc:
        with (
            tc.tile_pool(name="const", bufs=1) as constp,
            tc.tile_pool(name="norm", bufs=4) as normp,
            tc.tile_pool(name="psum", bufs=1, space="PSUM") as psump,
        ):
            stats2_t = constp.tile([H2, 2], F32)
            nc.sync.dma_start(out=stats2_t[:], in_=stats2[:, :])
            g2_t = constp.tile([H2, 1], F32)
            nc.sync.dma_start(out=g2_t[:], in_=g2[:, :])
            be2_t = constp.tile([H2, 1], F32)
            nc.sync.dma_start(out=be2_t[:], in_=be2[:, :])
            scale_full, shift_full = _emit_scale_shift(
                nc, constp, psump, stats2_t, g2_t, be2_t, H2, N_NODES)
            CH = 7
            hv = h2i[:, :].rearrange("(t p j) f -> t p (j f)", p=P, j=CH)
            ov = outo[:, :].rearrange("(t p j) f -> t p (j f)", p=P, j=CH)
            n_t = SLOTS // (P * CH)
            sc_b = scale_full[:].rearrange("p f -> p () f").to_broadcast(
                [P, CH, H2])
            sh_b = shift_full[:].rearrange("p f -> p () f").to_broadcast(
                [P, CH, H2])
            for t in range(n_t):
                nt = normp.tile([P, CH * H2], F32, tag="nt")
                nc.sync.dma_start(out=nt[:], in_=hv[t])
                sc = normp.tile([P, CH * H2], F32, tag="sc")
                nc.vector.tensor_tensor(
                    out=sc[:].rearrange("p (j f) -> p j f", f=H2),
                    in0=nt[:].rearrange("p (j f) -> p j f", f=H2),
                    in1=sc_b,
                    op=mybir.AluOpType.mult)
                nc.vector.tensor_tensor(
                    out=sc[:].rearrange("p (j f) -> p j f", f=H2),
                    in0=sc[:].rearrange("p (j f) -> p j f", f=H2),
                    in1=sh_b,
                    op=mybir.AluOpType.add)
                nc.scalar.activation(sc[:], sc[:],
                                     mybir.ActivationFunctionType.Relu)
                nc.sync.dma_start(out=ov[t], in_=sc[:])

    return nc


# ---------------------------------------------------------------- entry
_TRACE = {"enabled": False, "exec_ns": []}


def _run(nc, in_maps):
    _split_sync_waits(nc)
    kw = {}
    if _TRACE["enabled"]:
        kw["trace"] = True
    res = run_bass_kernel_spmd(nc, in_maps, core_ids=list(range(N_CORES)), **kw)
    if _TRACE["enabled"]:
        _TRACE["exec_ns"].append(res.exec_time_ns)
    return res.results


def kernel(**inputs):
    x = np.ascontiguousarray(
        np.vstack([np.asarray(inputs["x"], dtype=np.float32),
                   np.zeros((1, F_IN), np.float32)]))
    plan = _build_plan(np.asarray(inputs["edge_index"]))

    def col(v, n):
        return np.ascontiguousarray(
            np.asarray(v, dtype=np.float32).reshape(n, 1))

    W1a = np.ascontiguousarray(np.asarray(inputs["W1a"], np.float32))
    W1b = np.ascontiguousarray(np.asarray(inputs["W1b"], np.float32))
    W2a = np.ascontiguousarray(np.asarray(inputs["W2a"], np.float32))
    W2b = np.ascontiguousarray(np.asarray(inputs["W2b"], np.float32))

    nc1 = _build_launch1(plan)
    in1 = [
        {"x": x, "idx1": plan["cores"][c]["idx1"], "w1a": W1a,
         "b1a": col(inputs["b1a"], H1), "w1b": W1b,
         "b1b": col(inputs["b1b"], H1)}
        for c in range(N_CORES)
    ]
    res1 = _run(nc1, in1)
    tab1 = np.vstack([res1[c]["h1o"] for c in range(N_CORES)])
    stats1 = np.sum([res1[c]["stats"] for c in range(N_CORES)], axis=0)
    stats1 = np.ascontiguousarray(stats1.astype(np.float32))

    nc2 = _build_launch2(plan)
    in2 = [
        {"tab1": tab1, "stats1": stats1, "g1": col(inputs["g1"], H1),
         "be1": col(inputs["be1"], H1), "idx2": plan["cores"][c]["idx2"],
         "w2a": W2a, "b2a": col(inputs["b2a"], H2), "w2b": W2b,
         "b2b": col(inputs["b2b"], H2)}
        for c in range(N_CORES)
    ]
    res2 = _run(nc2, in2)
    stats2 = np.sum([res2[c]["stats"] for c in range(N_CORES)], axis=0)
    stats2 = np.ascontiguousarray(stats2.astype(np.float32))

    nc3 = _build_launch3()
    in3 = [
        {"h2i": res2[c]["h2o"], "stats2": stats2,
         "g2": col(inputs["g2"], H2), "be2": col(inputs["be2"], H2)}
        for c in range(N_CORES)
    ]
    res3 = _run(nc3, in3)

    out = np.zeros((N_NODES, H2), dtype=np.float32)
    for c in range(N_CORES):
        perm = plan["cores"][c]["perm"]
        out[c * PER_CORE + perm] = res3[c]["outo"][:PER_CORE]
    return out


# revision 12
# speedup vs baseline: 1.0312x; 1.0312x over previous
"""GIN 2-layer (GINConv + MLP + BN + ReLU) x2 on 8 Trainium2 NeuronCores.

Sharding: dst-shard the 100k nodes into 8 contiguous ranges of 12500
(edges partitioned by dst core). Within a core, nodes are permuted by
descending in-degree into 98 blocks of 128 slots (44 trailing pads).
Aggregation = per-block gather-accumulate "rounds": round r gathers the
r-th in-edge's source row for each of the block's 128 nodes via
indirect DMA with CCE-add into the SBUF accumulator (OOB sentinel
indices accumulate zero). The MLP runs in feature-on-partition
(transposed) layout so biases/BN are per-partition and BN stats are
free-axis reductions. BatchNorm needs global batch stats, so the work
is split into 3 device launches:
  L1: aggregate + MLP1 -> h1_pre shards (node-major) + per-core BN sums
  (host concatenates shards / adds partial sums -- data movement only)
  L2: BN1+ReLU over the full table, aggregate + MLP2 -> h2_pre shards + sums
  L3: BN2+ReLU -> output shards
"""
import sys

sys.path.insert(0, "/opt/trn_rl_repo")

import numpy as np
from concourse import bass, mybir
import concourse.tile as tile
from concourse.bass_utils import run_bass_kernel_spmd
from concourse.masks import make_identity

N_NODES = 100000
N_CORES = 8
P = 128
PER_CORE = N_NODES // N_CORES          # 12500
BLOCKS = (PER_CORE + P - 1) // P       # 98
SLOTS = BLOCKS * P                     # 12544
TABLE_ROWS = N_CORES * SLOTS           # 100352
LAST_VALID = PER_CORE - (BLOCKS - 1) * P   # 84 valid cols in last block
F_IN, H1, H2 = 256, 128, 32
BN_EPS = 1e-5
F32 = mybir.dt.float32
I32 = mybir.dt.int32


# ---------------------------------------------------------------- wait split
def _split_sync_waits(nc, max_waits=1):
    """This container's walrus rejects >1 sync wait per instruction; hoist
    extras onto preceding same-engine NoOps."""
    ctr = 0
    for f in nc.m.functions:
        for bb in f.blocks:
            out, changed = [], False
            for inst in list(bb.instructions):
                si = getattr(inst, "sync_info", None)
                if si is not None and si.on_wait and len(si.on_wait) > max_waits:
                    waits = list(si.on_wait)
                    extra, keep = waits[:-max_waits], waits[-max_waits:]
                    for i in range(0, len(extra), max_waits):
                        ctr += 1
                        out.append(mybir.InstNoOp(
                            name=f"waitsplit-nop-{ctr}", ins=[], outs=[],
                            engine=inst.engine,
                            sync_info=mybir.SyncInfo(
                                on_wait=extra[i:i + max_waits], on_update=[]),
                        ))
                    inst.sync_info = mybir.SyncInfo(
                        on_wait=keep, on_update=list(si.on_update or []))
                    changed = True
                out.append(inst)
            if changed:
                bb.instructions = out
    return ctr


# ---------------------------------------------------------------- host plan
def _build_plan(edge_index):
    src = np.asarray(edge_index[0], dtype=np.int64)
    dst = np.asarray(edge_index[1], dtype=np.int64)
    core_of = dst // PER_CORE

    per_core, all_R = [], np.zeros((N_CORES, BLOCKS), dtype=np.int64)
    for c in range(N_CORES):
        sel = np.nonzero(core_of == c)[0]
        s_c = src[sel]
        d_c = dst[sel] - c * PER_CORE
        deg = np.bincount(d_c, minlength=PER_CORE)
        perm = np.argsort(-deg, kind="stable")
        pos_of = np.empty(PER_CORE, dtype=np.int64)
        pos_of[perm] = np.arange(PER_CORE)
        slot = pos_of[d_c]
        order = np.argsort(slot, kind="stable")
        s_c, slot = s_c[order], slot[order]
        first = np.searchsorted(slot, np.arange(SLOTS), side="left")
        counts = np.bincount(slot, minlength=SLOTS)
        rank = np.arange(len(slot)) - first[slot]
        blk = slot // P
        all_R[c] = counts.reshape(BLOCKS, P).max(axis=1)
        per_core.append(dict(perm=perm, pos_of=pos_of, s=s_c, slot=slot,
                             rank=rank, blk=blk))

    R = all_R.max(axis=0)
    offs = np.zeros(BLOCKS + 1, dtype=np.int64)
    offs[1:] = np.cumsum(R)
    TOT = int(offs[-1])

    tab_pos = np.empty(N_NODES, dtype=np.int64)
    for c in range(N_CORES):
        ids = np.arange(c * PER_CORE, (c + 1) * PER_CORE)
        tab_pos[ids] = c * SLOTS + per_core[c]["pos_of"]

    # combined idx tables: per block b the columns are
    # [self][round 0]...[round R[b]-1]; block b starts at b + offs[b]
    TOTC = BLOCKS + TOT
    starts = (np.arange(BLOCKS) + offs[:-1]).astype(np.int64)
    cores = []
    for c in range(N_CORES):
        pc = per_core[c]
        col = starts[pc["blk"]] + 1 + pc["rank"]
        row = pc["slot"] % P

        idx1 = np.full((P, TOTC), N_NODES, dtype=np.int32)
        idx1[row, col] = pc["s"]
        sl = np.arange(SLOTS)
        gl = np.where(sl < PER_CORE,
                      np.concatenate([pc["perm"] + c * PER_CORE,
                                      np.zeros(SLOTS - PER_CORE, np.int64)])[
                          np.minimum(sl, SLOTS - 1)],
                      0)
        # self column for block b at starts[b]
        idx1[:, starts] = gl.reshape(BLOCKS, P).T.astype(np.int32)

        idx2 = np.full((P, TOTC), TABLE_ROWS, dtype=np.int32)
        idx2[row, col] = tab_pos[pc["s"]]
        self2 = np.where(sl < PER_CORE, c * SLOTS + sl, 0)
        idx2[:, starts] = self2.reshape(BLOCKS, P).T.astype(np.int32)

        cores.append(dict(idx1=idx1, idx2=idx2, perm=pc["perm"]))

    return dict(R=R, offs=offs, TOT=TOT, TOTC=TOTC, starts=starts, cores=cores,
                tab_pos=tab_pos)


# ---------------------------------------------------------------- launches
GATHER_D = 4  # accumulate-chain split factor (breaks WAW serialization)


def _gather_rounds(nc, tc, pools, tab_ap, idx_dram, b, start, nrounds, width):
    """Self + nrounds gathers, rotating over GATHER_D accumulator slots to
    break the per-tile WAW chain (measured ~19-22 ns/row chained vs ~12
    unchained), then one strided DVE reduce over the used slots.
    All indices are valid rows; pad slots point at a trailing zero row of
    the table (adds 0). No OOB/bounds path -- it wedges the device at
    scale in this toolchain."""
    idxp, accp = pools
    idx_t = idxp.tile([P, 1 + nrounds], I32)
    nc.sync.dma_start(out=idx_t[:], in_=idx_dram[:, start:start + 1 + nrounds])
    nused = min(1 + nrounds, GATHER_D)
    acc = accp.tile([P, nused * width], F32, tag="accw")
    for k in range(1 + nrounds):
        slot = k % GATHER_D
        nc.gpsimd.indirect_dma_start(
            out=acc[:, slot * width:(slot + 1) * width], out_offset=None,
            in_=tab_ap,
            in_offset=bass.IndirectOffsetOnAxis(ap=idx_t[:, k:k + 1], axis=0),
            compute_op=(mybir.AluOpType.bypass if k < GATHER_D
                        else mybir.AluOpType.add))
    if nused == 1:
        return acc
    accsum = accp.tile([P, width], F32, tag="accsum")
    nc.vector.tensor_reduce(
        out=accsum[:],
        in_=acc[:, 0:nused * width].rearrange("p (d w) -> p w d", w=width),
        axis=mybir.AxisListType.X, op=mybir.AluOpType.add)
    return accsum


def _build_launch1(plan):
    nc = bass.Bass("TRN2", target_bir_lowering=False, debug=False,
                   num_devices=N_CORES)
    x = nc.dram_tensor("x", [N_NODES + 1, F_IN], F32, kind="ExternalInput")
    idx1 = nc.dram_tensor("idx1", [P, plan["TOTC"]], I32, kind="ExternalInput")
    w1a = nc.dram_tensor("w1a", [F_IN, H1], F32, kind="ExternalInput")
    b1a = nc.dram_tensor("b1a", [H1, 1], F32, kind="ExternalInput")
    w1b = nc.dram_tensor("w1b", [H1, H1], F32, kind="ExternalInput")
    b1b = nc.dram_tensor("b1b", [H1, 1], F32, kind="ExternalInput")
    h1o = nc.dram_tensor("h1o", [SLOTS, H1], F32, kind="ExternalOutput")
    stats = nc.dram_tensor("stats", [H1, 2], F32, kind="ExternalOutput")

    offs, R, starts = plan["offs"], plan["R"], plan["starts"]
    with tile.TileContext(nc) as tc:
        with (
            tc.tile_pool(name="const", bufs=1) as constp,
            tc.tile_pool(name="idxp", bufs=4) as idxp,
            tc.tile_pool(name="accp", bufs=8) as accp,
            tc.tile_pool(name="work", bufs=4) as workp,
            tc.tile_pool(name="psum", bufs=1, space="PSUM") as psump,
        ):
            ident = constp.tile([P, P], F32)
            make_identity(nc, ident[:])
            w1a_t = constp.tile([P, 2 * H1], F32)   # [128, 256]: two K-chunks
            nc.sync.dma_start(out=w1a_t[:, 0:H1], in_=w1a[0:P, :])
            nc.sync.dma_start(out=w1a_t[:, H1:2 * H1], in_=w1a[P:2 * P, :])
            w1b_t = constp.tile([H1, H1], F32)
            nc.sync.dma_start(out=w1b_t[:], in_=w1b[:, :])
            b1a_t = constp.tile([H1, 1], F32)
            nc.sync.dma_start(out=b1a_t[:], in_=b1a[:, :])
            b1b_t = constp.tile([H1, 1], F32)
            nc.sync.dma_start(out=b1b_t[:], in_=b1b[:, :])
            mask = constp.tile([P, P], F32)
            nc.vector.memset(mask[:], 1.0)
            nc.vector.memset(mask[:, LAST_VALID:], 0.0)
            s_sum = constp.tile([H1, 1], F32)
            s_sq = constp.tile([H1, 1], F32)
            nc.vector.memset(s_sum[:], 0.0)
            nc.vector.memset(s_sq[:], 0.0)

            for b in range(BLOCKS):
                acc = _gather_rounds(nc, tc, (idxp, accp), x[:, :], idx1,
                                     b, int(starts[b]), int(R[b]), F_IN)
                # transpose acc -> accT (2 x [128,128])
                accT = workp.tile([P, F_IN], F32, tag="accT")
                for cchunk in range(2):
                    pT = psump.tile([P, P], F32, tag="pT", space="PSUM")
                    nc.tensor.transpose(
                        out=pT[:], in_=acc[:, cchunk * P:(cchunk + 1) * P],
                        identity=ident[:])
                    nc.vector.tensor_copy(
                        out=accT[:, cchunk * P:(cchunk + 1) * P], in_=pT[:])
                ps1 = psump.tile([H1, P], F32, tag="ps1", space="PSUM")
                nc.tensor.matmul(out=ps1[:], lhsT=w1a_t[:, 0:H1],
                                 rhs=accT[:, 0:P], start=True, stop=False)
                nc.tensor.matmul(out=ps1[:], lhsT=w1a_t[:, H1:2 * H1],
                                 rhs=accT[:, P:2 * P], start=False, stop=True)
                r1 = workp.tile([H1, P], F32, tag="r1")
                nc.scalar.activation(r1[:], ps1[:],
                                     mybir.ActivationFunctionType.Relu,
                                     bias=b1a_t[:, 0:1])
                ps2 = psump.tile([H1, P], F32, tag="ps2", space="PSUM")
                nc.tensor.matmul(out=ps2[:], lhsT=w1b_t[:], rhs=r1[:],
                                 start=True, stop=True)
                hpre = workp.tile([H1, P], F32, tag="hpre")
                nc.scalar.activation(hpre[:], ps2[:],
                                     mybir.ActivationFunctionType.Identity,
                                     bias=b1b_t[:, 0:1])
                # stats (mask the pad columns of the last block)
                if b == BLOCKS - 1:
                    hstat = workp.tile([H1, P], F32, tag="hstat")
                    nc.vector.tensor_tensor(out=hstat[:], in0=hpre[:],
                                            in1=mask[0:H1, :],
                                            op=mybir.AluOpType.mult)
                else:
                    hstat = hpre
                part = workp.tile([H1, 1], F32, tag="part")
                nc.vector.tensor_reduce(out=part[:], in_=hstat[:],
                                        axis=mybir.AxisListType.X,
                                        op=mybir.AluOpType.add)
                nc.vector.tensor_add(out=s_sum[:], in0=s_sum[:], in1=part[:])
                sq = workp.tile([H1, P], F32, tag="sq")
                nc.scalar.square(sq[:], hstat[:])
                part2 = workp.tile([H1, 1], F32, tag="part2")
                nc.vector.tensor_reduce(out=part2[:], in_=sq[:],
                                        axis=mybir.AxisListType.X,
                                        op=mybir.AluOpType.add)
                nc.vector.tensor_add(out=s_sq[:], in0=s_sq[:], in1=part2[:])
                # store node-major
                psT = psump.tile([P, H1], F32, tag="psT", space="PSUM")
                nc.tensor.transpose(out=psT[:], in_=hpre[:],
                                    identity=ident[0:H1, 0:H1])
                hrow = workp.tile([P, H1], F32, tag="hrow")
                nc.vector.tensor_copy(out=hrow[:], in_=psT[:])
                nc.sync.dma_start(out=h1o[b * P:(b + 1) * P, :], in_=hrow[:])

            nc.sync.dma_start(out=stats[:, 0:1], in_=s_sum[:])
            nc.sync.dma_start(out=stats[:, 1:2], in_=s_sq[:])

    return nc


def _emit_scale_shift(nc, constp, psump, statsc, g, be, nf, n_count):
    """Device BN coefficient computation from raw sums.
    statsc [nf,2] (sum, sumsq); returns full [P, nf] scale/shift tiles."""
    mean = constp.tile([nf, 1], F32)
    nc.scalar.mul(mean[:], statsc[:, 0:1], 1.0 / n_count)
    ex2 = constp.tile([nf, 1], F32)
    nc.scalar.mul(ex2[:], statsc[:, 1:2], 1.0 / n_count)
    msq = constp.tile([nf, 1], F32)
    nc.vector.tensor_tensor(out=msq[:], in0=mean[:], in1=mean[:],
                            op=mybir.AluOpType.mult)
    var = constp.tile([nf, 1], F32)
    nc.vector.tensor_tensor(out=var[:], in0=ex2[:], in1=msq[:],
                            op=mybir.AluOpType.subtract)
    veps = constp.tile([nf, 1], F32)
    nc.vector.tensor_scalar_add(veps[:], var[:], BN_EPS)
    std = constp.tile([nf, 1], F32)
    nc.scalar.sqrt(std[:], veps[:])
    istd = constp.tile([nf, 1], F32)
    nc.vector.reciprocal(istd[:], std[:])
    scale_c = constp.tile([nf, 1], F32)
    nc.vector.tensor_tensor(out=scale_c[:], in0=g[:], in1=istd[:],
                            op=mybir.AluOpType.mult)
    mscale = constp.tile([nf, 1], F32)
    nc.vector.tensor_tensor(out=mscale[:], in0=mean[:], in1=scale_c[:],
                            op=mybir.AluOpType.mult)
    shift_c = constp.tile([nf, 1], F32)
    nc.vector.tensor_tensor(out=shift_c[:], in0=be[:], in1=mscale[:],
                            op=mybir.AluOpType.subtract)
    # broadcast to [P, nf] via rank-1 matmul: ones_row^T (x) coef_row
    ident1 = constp.tile([nf, nf], F32)
    make_identity(nc, ident1[:])
    ones_row = constp.tile([1, P], F32)
    nc.vector.memset(ones_row[:], 1.0)
    out_tiles = []
    for i, coef in enumerate((scale_c, shift_c)):
        prow_ps = psump.tile([1, nf], F32, tag=f"prow{i}", space="PSUM")
        nc.tensor.transpose(out=prow_ps[:], in_=coef[:],
                            identity=ident1[0:nf, 0:nf])
        crow = constp.tile([1, nf], F32, tag=f"crow{i}")
        nc.vector.tensor_copy(out=crow[:], in_=prow_ps[:])
        full_ps = psump.tile([P, nf], F32, tag=f"fullp{i}", space="PSUM")
        nc.tensor.matmul(out=full_ps[:], lhsT=ones_row[:], rhs=crow[:],
                         start=True, stop=True)
        full = constp.tile([P, nf], F32, tag=f"coef_full{i}")
        nc.vector.tensor_copy(out=full[:], in_=full_ps[:])
        out_tiles.append(full)
    return out_tiles


def _build_launch2(plan):
    nc = bass.Bass("TRN2", target_bir_lowering=False, debug=False,
                   num_devices=N_CORES)
    tab1 = nc.dram_tensor("tab1", [TABLE_ROWS, H1], F32, kind="ExternalInput")
    stats1 = nc.dram_tensor("stats1", [H1, 2], F32, kind="ExternalInput")
    g1 = nc.dram_tensor("g1", [H1, 1], F32, kind="ExternalInput")
    be1 = nc.dram_tensor("be1", [H1, 1], F32, kind="ExternalInput")
    idx2 = nc.dram_tensor("idx2", [P, plan["TOTC"]], I32, kind="ExternalInput")
    w2a = nc.dram_tensor("w2a", [H1, H2], F32, kind="ExternalInput")
    b2a = nc.dram_tensor("b2a", [H2, 1], F32, kind="ExternalInput")
    w2b = nc.dram_tensor("w2b", [H2, H2], F32, kind="ExternalInput")
    b2b = nc.dram_tensor("b2b", [H2, 1], F32, kind="ExternalInput")
    h2o = nc.dram_tensor("h2o", [SLOTS, H2], F32, kind="ExternalOutput")
    stats = nc.dram_tensor("stats", [H2, 2], F32, kind="ExternalOutput")
    relu_tab = nc.dram_tensor("relu_tab", [TABLE_ROWS + 1, H1], F32,
                              kind="Internal")

    offs, R, starts = plan["offs"], plan["R"], plan["starts"]
    # Phase 1 (own TileContext => hard barrier before phase 2): BN1 coeffs
    # + normalize/relu the full table into relu_tab.
    with tile.TileContext(nc) as tc:
        with (
            tc.tile_pool(name="const1", bufs=1) as constp,
            tc.tile_pool(name="norm", bufs=4) as normp,
            tc.tile_pool(name="psum1", bufs=1, space="PSUM") as psump,
        ):
            stats1_t = constp.tile([H1, 2], F32)
            nc.sync.dma_start(out=stats1_t[:], in_=stats1[:, :])
            g1_t = constp.tile([H1, 1], F32)
            nc.sync.dma_start(out=g1_t[:], in_=g1[:, :])
            be1_t = constp.tile([H1, 1], F32)
            nc.sync.dma_start(out=be1_t[:], in_=be1[:, :])
            scale_full, shift_full = _emit_scale_shift(
                nc, constp, psump, stats1_t, g1_t, be1_t, H1, N_NODES)

            # normalize + relu the full table: 98 strided tiles of 1024 rows
            CH = 8  # rows per partition per tile
            tabv = tab1[:, :].rearrange("(t p j) f -> t p (j f)", p=P, j=CH)
            reluv = relu_tab[0:TABLE_ROWS, :].rearrange("(t p j) f -> t p (j f)", p=P, j=CH)
            n_t = TABLE_ROWS // (P * CH)
            sc_b = scale_full[:].rearrange("p f -> p () f").to_broadcast(
                [P, CH, H1])
            sh_b = shift_full[:].rearrange("p f -> p () f").to_broadcast(
                [P, CH, H1])
            for t in range(n_t):
                nt = normp.tile([P, CH * H1], F32, tag="nt")
                nc.sync.dma_start(out=nt[:], in_=tabv[t])
                sc = normp.tile([P, CH * H1], F32, tag="sc")
                nc.vector.tensor_tensor(
                    out=sc[:].rearrange("p (j f) -> p j f", f=H1),
                    in0=nt[:].rearrange("p (j f) -> p j f", f=H1),
                    in1=sc_b,
                    op=mybir.AluOpType.mult)
                nc.vector.tensor_tensor(
                    out=sc[:].rearrange("p (j f) -> p j f", f=H1),
                    in0=sc[:].rearrange("p (j f) -> p j f", f=H1),
                    in1=sh_b,
                    op=mybir.AluOpType.add)
                nc.scalar.activation(sc[:], sc[:],
                                     mybir.ActivationFunctionType.Relu)
                nc.sync.dma_start(out=reluv[t], in_=sc[:])
            zrow = normp.tile([1, H1], F32, tag="zrow")
            nc.vector.memset(zrow[:], 0.0)
            nc.sync.dma_start(out=relu_tab[TABLE_ROWS:TABLE_ROWS + 1, :],
                              in_=zrow[:])

    # Phase 2: aggregate from relu_tab + MLP2 + stats.
    with tile.TileContext(nc) as tc:
        with (
            tc.tile_pool(name="const", bufs=1) as constp,
            tc.tile_pool(name="idxp", bufs=4) as idxp,
            tc.tile_pool(name="accp", bufs=8) as accp,
            tc.tile_pool(name="work", bufs=4) as workp,
            tc.tile_pool(name="psum", bufs=1, space="PSUM") as psump,
        ):
            ident = constp.tile([P, P], F32)
            make_identity(nc, ident[:])
            # layer-2 weights/consts
            w2a_t = constp.tile([H1, H2], F32)
            nc.sync.dma_start(out=w2a_t[:], in_=w2a[:, :])
            w2b_t = constp.tile([H2, H2], F32)
            nc.sync.dma_start(out=w2b_t[:], in_=w2b[:, :])
            b2a_t = constp.tile([H2, 1], F32)
            nc.sync.dma_start(out=b2a_t[:], in_=b2a[:, :])
            b2b_t = constp.tile([H2, 1], F32)
            nc.sync.dma_start(out=b2b_t[:], in_=b2b[:, :])
            mask = constp.tile([H2, P], F32)
            nc.vector.memset(mask[:], 1.0)
            nc.vector.memset(mask[:, LAST_VALID:], 0.0)
            s_sum = constp.tile([H2, 1], F32)
            s_sq = constp.tile([H2, 1], F32)
            nc.vector.memset(s_sum[:], 0.0)
            nc.vector.memset(s_sq[:], 0.0)

            for b in range(BLOCKS):
                acc = _gather_rounds(nc, tc, (idxp, accp), relu_tab[:, :],
                                     idx2, b, int(starts[b]),
                                     int(R[b]), H1)
                accT = workp.tile([P, P], F32, tag="accT")
                pT = psump.tile([P, P], F32, tag="pT", space="PSUM")
                nc.tensor.transpose(out=pT[:], in_=acc[:], identity=ident[:])
                nc.vector.tensor_copy(out=accT[:], in_=pT[:])
                ps1 = psump.tile([H2, P], F32, tag="ps1", space="PSUM")
                nc.tensor.matmul(out=ps1[:], lhsT=w2a_t[:], rhs=accT[:],
                                 start=True, stop=True)
                r1 = workp.tile([H2, P], F32, tag="r1")
                nc.scalar.activation(r1[:], ps1[:],
                                     mybir.ActivationFunctionType.Relu,
                                     bias=b2a_t[:, 0:1])
                ps2 = psump.tile([H2, P], F32, tag="ps2", space="PSUM")
                nc.tensor.matmul(out=ps2[:], lhsT=w2b_t[:], rhs=r1[:],
                                 start=True, stop=True)
                hpre = workp.tile([H2, P], F32, tag="hpre")
                nc.scalar.activation(hpre[:], ps2[:],
                                     mybir.ActivationFunctionType.Identity,
                                     bias=b2b_t[:, 0:1])
                if b == BLOCKS - 1:
                    hstat = workp.tile([H2, P], F32, tag="hstat")
                    nc.vector.tensor_tensor(out=hstat[:], in0=hpre[:],
                                            in1=mask[:],
                                            op=mybir.AluOpType.mult)
                else:
                    hstat = hpre
                part = workp.tile([H2, 1], F32, tag="part")
                nc.vector.tensor_reduce(out=part[:], in_=hstat[:],
                                        axis=mybir.AxisListType.X,
                                        op=mybir.AluOpType.add)
                nc.vector.tensor_add(out=s_sum[:], in0=s_sum[:], in1=part[:])
                sq = workp.tile([H2, P], F32, tag="sq")
                nc.scalar.square(sq[:], hstat[:])
                part2 = workp.tile([H2, 1], F32, tag="part2")
                nc.vector.tensor_reduce(out=part2[:], in_=sq[:],
                                        axis=mybir.AxisListType.X,
                                        op=mybir.AluOpType.add)
                nc.vector.tensor_add(out=s_sq[:], in0=s_sq[:], in1=part2[:])
                psT = psump.tile([P, H2], F32, tag="psT", space="PSUM")
                nc.tensor.transpose(out=psT[:], in_=hpre[:],
                                    identity=ident[0:H2, 0:H2])
                hrow = workp.tile([P, H2], F32, tag="hrow")
                nc.vector.tensor_copy(out=hrow[:], in_=psT[:])
                nc.sync.dma_start(out=h2o[b * P:(b + 1) * P, :], in_=hrow[:])

            nc.sync.dma_start(out=stats[:, 0:1], in_=s_sum[:])
            nc.sync.dma_start(out=stats[:, 1:2], in_=s_sq[:])

    return nc


def _build_launch3():
    nc = bass.Bass("TRN2", target_bir_lowering=False, debug=False,
                   num_devices=N_CORES)
    h2i = nc.dram_tensor("h2i", [SLOTS, H2], F32, kind="ExternalInput")
    stats2 = nc.dram_tensor("stats2", [H2, 2], F32, kind="ExternalInput")
    g2 = nc.dram_tensor("g2", [H2, 1], F32, kind="ExternalInput")
    be2 = nc.dram_tensor("be2", [H2, 1], F32, kind="ExternalInput")
    outo = nc.dram_tensor("outo", [SLOTS, H2], F32, kind="ExternalOutput")

    with tile.TileContext(nc) as tc:
        with (
            tc.tile_pool(name="const", bufs=1) as constp,
            tc.tile_pool(name="norm", bufs=4) as normp,
            tc.tile_pool(name="psum", bufs=1, space="PSUM") as psump,
        ):
            stats2_t = constp.tile([H2, 2], F32)
            nc.sync.dma_start(out=stats2_t[:], in_=stats2[:, :])
            g2_t = constp.tile([H2, 1], F32)
            nc.sync.dma_start(out=g2_t[:], in_=g2[:, :])
            be2_t = constp.tile([H2, 1], F32)
            nc.sync.dma_start(out=be2_t[:], in_=be2[:, :])
            scale_full, shift_full = _emit_scale_shift(
                nc, constp, psump, stats2_t, g2_t, be2_t, H2, N_NODES)
            CH = 7
            hv = h2i[:, :].rearrange("(t p j) f -> t p (j f)", p=P, j=CH)
            ov = outo[:, :].rearrange("(t p j) f -> t p (j f)", p=P, j=CH)
            n_t = SLOTS // (P * CH)
            sc_b = scale_full[:].rearrange("p f -> p () f").to_broadcast(
                [P, CH, H2])
            sh_b = shift_full[:].rearrange("p f -> p () f").to_broadcast(
                [P, CH, H2])
            for t in range(n_t):
                nt = normp.tile([P, CH * H2], F32, tag="nt")
                nc.sync.dma_start(out=nt[:], in_=hv[t])
                sc = normp.tile([P, CH * H2], F32, tag="sc")
                nc.vector.tensor_tensor(
                    out=sc[:].rearrange("p (j f) -> p j f", f=H2),
                    in0=nt[:].rearrange("p (j f) -> p j f", f=H2),
                    in1=sc_b,
                    op=mybir.AluOpType.mult)
                nc.vector.tensor_tensor(
                    out=sc[:].rearrange("p (j f) -> p j f", f=H2),
                    in0=sc[:].rearrange("p (j f) -> p j f", f=H2),
                    in1=sh_b,
                    op=mybir.AluOpType.add)
                nc.scalar.activation(sc[:], sc[:],
                                     mybir.ActivationFunctionType.Relu)
                nc.sync.dma_start(out=ov[t], in_=sc[:])

    return nc


# ---------------------------------------------------------------- entry
_TRACE = {"enabled": False, "exec_ns": []}


def _run(nc, in_maps):
    _split_sync_waits(nc)
    kw = {}
    if _TRACE["enabled"]:
        kw["trace"] = True
    res = run_bass_kernel_spmd(nc, in_maps, core_ids=list(range(N_CORES)), **kw)
    if _TRACE["enabled"]:
        _TRACE["exec_ns"].append(res.exec_time_ns)
    return res.results


def kernel(**inputs):
    x = np.ascontiguousarray(
        np.vstack([np.asarray(inputs["x"], dtype=np.float32),
                   np.zeros((1, F_IN), np.float32)]))
    plan = _build_plan(np.asarray(inputs["edge_index"]))

    def col(v, n):
        return np.ascontiguousarray(
            np.asarray(v, dtype=np.float32).reshape(n, 1))

    W1a = np.ascontiguousarray(np.asarray(inputs["W1a"], np.float32))
    W1b = np.ascontiguousarray(np.asarray(inputs["W1b"], np.float32))
    W2a = np.ascontiguousarray(np.asarray(inputs["W2a"], np.float32))
    W2b = np.ascontiguousarray(np.asarray(inputs["W2b"], np.float32))

    nc1 = _build_launch1(plan)
    in1 = [
        {"x": x, "idx1": plan["cores"][c]["idx1"], "w1a": W1a,
         "b1a": col(inputs["b1a"], H1), "w1b": W1b,
         "b1b": col(inputs["b1b"], H1)}
        for c in range(N_CORES)
    ]
    res1 = _run(nc1, in1)
    tab1 = np.vstack([res1[c]["h1o"] for c in range(N_CORES)])
    stats1 = np.sum([res1[c]["stats"] for c in range(N_CORES)], axis=0)
    stats1 = np.ascontiguousarray(stats1.astype(np.float32))

    nc2 = _build_launch2(plan)
    in2 = [
        {"tab1": tab1, "stats1": stats1, "g1": col(inputs["g1"], H1),
         "be1": col(inputs["be1"], H1), "idx2": plan["cores"][c]["idx2"],
         "w2a": W2a, "b2a": col(inputs["b2a"], H2), "w2b": W2b,
         "b2b": col(inputs["b2b"], H2)}
        for c in range(N_CORES)
    ]
    res2 = _run(nc2, in2)
    stats2 = np.sum([res2[c]["stats"] for c in range(N_CORES)], axis=0)
    stats2 = np.ascontiguousarray(stats2.astype(np.float32))

    nc3 = _build_launch3()
    in3 = [
        {"h2i": res2[c]["h2o"], "stats2": stats2,
         "g2": col(inputs["g2"], H2), "be2": col(inputs["be2"], H2)}
        for c in range(N_CORES)
    ]
    res3 = _run(nc3, in3)

    out = np.zeros((N_NODES, H2), dtype=np.float32)
    for c in range(N_CORES):
        perm = plan["cores"][c]["perm"]
        out[c * PER_CORE + perm] = res3[c]["outo"][:PER_CORE]
    return out


# revision 14
# speedup vs baseline: 1.0733x; 1.0408x over previous
"""GIN 2-layer (GINConv + MLP + BN + ReLU) x2 on 8 Trainium2 NeuronCores.

Sharding: dst-shard the 100k nodes into 8 contiguous ranges of 12500
(edges partitioned by dst core). Within a core, nodes are permuted by
descending in-degree into 98 blocks of 128 slots (44 trailing pads).
Aggregation = per-block gather-accumulate "rounds": round r gathers the
r-th in-edge's source row for each of the block's 128 nodes via
indirect DMA with CCE-add into SBUF accumulator slots (pad slots point
at a trailing zero row of the table, adding 0; rounds rotate over
GATHER_D slots to break the WAW chain, then one strided DVE reduce). The MLP runs in feature-on-partition
(transposed) layout so biases/BN are per-partition and BN stats are
free-axis reductions. BatchNorm needs global batch stats, so the work
is split into 3 device launches:
  L1: aggregate + MLP1 -> h1_pre shards (node-major) + per-core BN sums
  (host concatenates shards / adds partial sums -- data movement only)
  L2: BN1+ReLU over the full table, aggregate + MLP2 -> h2_pre shards + sums
  L3: BN2+ReLU -> output shards
"""
import sys

sys.path.insert(0, "/opt/trn_rl_repo")

import numpy as np
from concourse import bass, mybir
import concourse.tile as tile
from concourse.bass_utils import run_bass_kernel_spmd
from concourse.masks import make_identity

N_NODES = 100000
N_CORES = 8
P = 128
PER_CORE = N_NODES // N_CORES          # 12500
BLOCKS = (PER_CORE + P - 1) // P       # 98
SLOTS = BLOCKS * P                     # 12544
TABLE_ROWS = N_CORES * SLOTS           # 100352
LAST_VALID = PER_CORE - (BLOCKS - 1) * P   # 84 valid cols in last block
F_IN, H1, H2 = 256, 128, 32
BN_EPS = 1e-5
F32 = mybir.dt.float32
I32 = mybir.dt.int32


# ---------------------------------------------------------------- wait split
def _split_sync_waits(nc, max_waits=1):
    """This container's walrus rejects >1 sync wait per instruction; hoist
    extras onto preceding same-engine NoOps."""
    ctr = 0
    for f in nc.m.functions:
        for bb in f.blocks:
            out, changed = [], False
            for inst in list(bb.instructions):
                si = getattr(inst, "sync_info", None)
                if si is not None and si.on_wait and len(si.on_wait) > max_waits:
                    waits = list(si.on_wait)
                    extra, keep = waits[:-max_waits], waits[-max_waits:]
                    for i in range(0, len(extra), max_waits):
                        ctr += 1
                        out.append(mybir.InstNoOp(
                            name=f"waitsplit-nop-{ctr}", ins=[], outs=[],
                            engine=inst.engine,
                            sync_info=mybir.SyncInfo(
                                on_wait=extra[i:i + max_waits], on_update=[]),
                        ))
                    inst.sync_info = mybir.SyncInfo(
                        on_wait=keep, on_update=list(si.on_update or []))
                    changed = True
                out.append(inst)
            if changed:
                bb.instructions = out
    return ctr


# ---------------------------------------------------------------- host plan
def _build_plan(edge_index):
    src = np.asarray(edge_index[0], dtype=np.int64)
    dst = np.asarray(edge_index[1], dtype=np.int64)
    core_of = dst // PER_CORE

    per_core, all_R = [], np.zeros((N_CORES, BLOCKS), dtype=np.int64)
    for c in range(N_CORES):
        sel = np.nonzero(core_of == c)[0]
        s_c = src[sel]
        d_c = dst[sel] - c * PER_CORE
        deg = np.bincount(d_c, minlength=PER_CORE)
        perm = np.argsort(-deg, kind="stable")
        pos_of = np.empty(PER_CORE, dtype=np.int64)
        pos_of[perm] = np.arange(PER_CORE)
        slot = pos_of[d_c]
        order = np.argsort(slot, kind="stable")
        s_c, slot = s_c[order], slot[order]
        first = np.searchsorted(slot, np.arange(SLOTS), side="left")
        counts = np.bincount(slot, minlength=SLOTS)
        rank = np.arange(len(slot)) - first[slot]
        blk = slot // P
        all_R[c] = counts.reshape(BLOCKS, P).max(axis=1)
        per_core.append(dict(perm=perm, pos_of=pos_of, s=s_c, slot=slot,
                             rank=rank, blk=blk))

    R = all_R.max(axis=0)
    offs = np.zeros(BLOCKS + 1, dtype=np.int64)
    offs[1:] = np.cumsum(R)
    TOT = int(offs[-1])

    tab_pos = np.empty(N_NODES, dtype=np.int64)
    for c in range(N_CORES):
        ids = np.arange(c * PER_CORE, (c + 1) * PER_CORE)
        tab_pos[ids] = c * SLOTS + per_core[c]["pos_of"]

    # combined idx tables: per block b the columns are
    # [self][round 0]...[round R[b]-1]; block b starts at b + offs[b]
    TOTC = BLOCKS + TOT
    starts = (np.arange(BLOCKS) + offs[:-1]).astype(np.int64)
    cores = []
    for c in range(N_CORES):
        pc = per_core[c]
        col = starts[pc["blk"]] + 1 + pc["rank"]
        row = pc["slot"] % P

        idx1 = np.full((P, TOTC), N_NODES, dtype=np.int32)
        idx1[row, col] = pc["s"]
        sl = np.arange(SLOTS)
        gl = np.where(sl < PER_CORE,
                      np.concatenate([pc["perm"] + c * PER_CORE,
                                      np.zeros(SLOTS - PER_CORE, np.int64)])[
                          np.minimum(sl, SLOTS - 1)],
                      0)
        # self column for block b at starts[b]
        idx1[:, starts] = gl.reshape(BLOCKS, P).T.astype(np.int32)

        idx2 = np.full((P, TOTC), TABLE_ROWS, dtype=np.int32)
        idx2[row, col] = tab_pos[pc["s"]]
        self2 = np.where(sl < PER_CORE, c * SLOTS + sl, 0)
        idx2[:, starts] = self2.reshape(BLOCKS, P).T.astype(np.int32)

        cores.append(dict(idx1=idx1, idx2=idx2, perm=pc["perm"]))

    return dict(R=R, offs=offs, TOT=TOT, TOTC=TOTC, starts=starts, cores=cores,
                tab_pos=tab_pos)


# ---------------------------------------------------------------- launches
GATHER_D = 8  # accumulate-chain split factor (breaks WAW serialization)


def _gather_rounds(nc, tc, pools, tab_ap, idx_dram, b, start, nrounds, width):
    """Self + nrounds gathers, rotating over GATHER_D accumulator slots to
    break the per-tile WAW chain (measured ~19-22 ns/row chained vs ~12
    unchained), then one strided DVE reduce over the used slots.
    All indices are valid rows; pad slots point at a trailing zero row of
    the table (adds 0). No OOB/bounds path -- it wedges the device at
    scale in this toolchain."""
    idxp, accp = pools
    idx_t = idxp.tile([P, 1 + nrounds], I32)
    nc.sync.dma_start(out=idx_t[:], in_=idx_dram[:, start:start + 1 + nrounds])
    nused = min(1 + nrounds, GATHER_D)
    acc = accp.tile([P, nused * width], F32, tag="accw")
    for k in range(1 + nrounds):
        slot = k % GATHER_D
        nc.gpsimd.indirect_dma_start(
            out=acc[:, slot * width:(slot + 1) * width], out_offset=None,
            in_=tab_ap,
            in_offset=bass.IndirectOffsetOnAxis(ap=idx_t[:, k:k + 1], axis=0),
            compute_op=(mybir.AluOpType.bypass if k < GATHER_D
                        else mybir.AluOpType.add))
    if nused == 1:
        return acc
    accsum = accp.tile([P, width], F32, tag="accsum")
    nc.vector.tensor_reduce(
        out=accsum[:],
        in_=acc[:, 0:nused * width].rearrange("p (d w) -> p w d", w=width),
        axis=mybir.AxisListType.X, op=mybir.AluOpType.add)
    return accsum


def _build_launch1(plan):
    nc = bass.Bass("TRN2", target_bir_lowering=False, debug=False,
                   num_devices=N_CORES)
    x = nc.dram_tensor("x", [N_NODES + 1, F_IN], F32, kind="ExternalInput")
    idx1 = nc.dram_tensor("idx1", [P, plan["TOTC"]], I32, kind="ExternalInput")
    w1a = nc.dram_tensor("w1a", [F_IN, H1], F32, kind="ExternalInput")
    b1a = nc.dram_tensor("b1a", [H1, 1], F32, kind="ExternalInput")
    w1b = nc.dram_tensor("w1b", [H1, H1], F32, kind="ExternalInput")
    b1b = nc.dram_tensor("b1b", [H1, 1], F32, kind="ExternalInput")
    h1o = nc.dram_tensor("h1o", [SLOTS, H1], F32, kind="ExternalOutput")
    stats = nc.dram_tensor("stats", [H1, 2], F32, kind="ExternalOutput")

    offs, R, starts = plan["offs"], plan["R"], plan["starts"]
    with tile.TileContext(nc) as tc:
        with (
            tc.tile_pool(name="const", bufs=1) as constp,
            tc.tile_pool(name="idxp", bufs=8) as idxp,
            tc.tile_pool(name="accp", bufs=12) as accp,
            tc.tile_pool(name="work", bufs=6) as workp,
            tc.tile_pool(name="psum", bufs=1, space="PSUM") as psump,
        ):
            ident = constp.tile([P, P], F32)
            make_identity(nc, ident[:])
            w1a_t = constp.tile([P, 2 * H1], F32)   # [128, 256]: two K-chunks
            nc.sync.dma_start(out=w1a_t[:, 0:H1], in_=w1a[0:P, :])
            nc.sync.dma_start(out=w1a_t[:, H1:2 * H1], in_=w1a[P:2 * P, :])
            w1b_t = constp.tile([H1, H1], F32)
            nc.sync.dma_start(out=w1b_t[:], in_=w1b[:, :])
            b1a_t = constp.tile([H1, 1], F32)
            nc.sync.dma_start(out=b1a_t[:], in_=b1a[:, :])
            b1b_t = constp.tile([H1, 1], F32)
            nc.sync.dma_start(out=b1b_t[:], in_=b1b[:, :])
            mask = constp.tile([P, P], F32)
            nc.vector.memset(mask[:], 1.0)
            nc.vector.memset(mask[:, LAST_VALID:], 0.0)
            s_sum = constp.tile([H1, 1], F32)
            s_sq = constp.tile([H1, 1], F32)
            nc.vector.memset(s_sum[:], 0.0)
            nc.vector.memset(s_sq[:], 0.0)

            for b in range(BLOCKS):
                acc = _gather_rounds(nc, tc, (idxp, accp), x[:, :], idx1,
                                     b, int(starts[b]), int(R[b]), F_IN)
                # transpose acc -> accT (2 x [128,128])
                accT = workp.tile([P, F_IN], F32, tag="accT")
                for cchunk in range(2):
                    pT = psump.tile([P, P], F32, tag="pT", space="PSUM")
                    nc.tensor.transpose(
                        out=pT[:], in_=acc[:, cchunk * P:(cchunk + 1) * P],
                        identity=ident[:])
                    nc.vector.tensor_copy(
                        out=accT[:, cchunk * P:(cchunk + 1) * P], in_=pT[:])
                ps1 = psump.tile([H1, P], F32, tag="ps1", space="PSUM")
                nc.tensor.matmul(out=ps1[:], lhsT=w1a_t[:, 0:H1],
                                 rhs=accT[:, 0:P], start=True, stop=False)
                nc.tensor.matmul(out=ps1[:], lhsT=w1a_t[:, H1:2 * H1],
                                 rhs=accT[:, P:2 * P], start=False, stop=True)
                r1 = workp.tile([H1, P], F32, tag="r1")
                nc.scalar.activation(r1[:], ps1[:],
                                     mybir.ActivationFunctionType.Relu,
                                     bias=b1a_t[:, 0:1])
                ps2 = psump.tile([H1, P], F32, tag="ps2", space="PSUM")
                nc.tensor.matmul(out=ps2[:], lhsT=w1b_t[:], rhs=r1[:],
                                 start=True, stop=True)
                hpre = workp.tile([H1, P], F32, tag="hpre")
                nc.scalar.activation(hpre[:], ps2[:],
                                     mybir.ActivationFunctionType.Identity,
                                     bias=b1b_t[:, 0:1])
                # stats (mask the pad columns of the last block)
                if b == BLOCKS - 1:
                    hstat = workp.tile([H1, P], F32, tag="hstat")
                    nc.vector.tensor_tensor(out=hstat[:], in0=hpre[:],
                                            in1=mask[0:H1, :],
                                            op=mybir.AluOpType.mult)
                else:
                    hstat = hpre
                part = workp.tile([H1, 1], F32, tag="part")
                nc.vector.tensor_reduce(out=part[:], in_=hstat[:],
                                        axis=mybir.AxisListType.X,
                                        op=mybir.AluOpType.add)
                nc.vector.tensor_add(out=s_sum[:], in0=s_sum[:], in1=part[:])
                sq = workp.tile([H1, P], F32, tag="sq")
                nc.scalar.square(sq[:], hstat[:])
                part2 = workp.tile([H1, 1], F32, tag="part2")
                nc.vector.tensor_reduce(out=part2[:], in_=sq[:],
                                        axis=mybir.AxisListType.X,
                                        op=mybir.AluOpType.add)
                nc.vector.tensor_add(out=s_sq[:], in0=s_sq[:], in1=part2[:])
                # store node-major
                psT = psump.tile([P, H1], F32, tag="psT", space="PSUM")
                nc.tensor.transpose(out=psT[:], in_=hpre[:],
                                    identity=ident[0:H1, 0:H1])
                hrow = workp.tile([P, H1], F32, tag="hrow")
                nc.vector.tensor_copy(out=hrow[:], in_=psT[:])
                nc.sync.dma_start(out=h1o[b * P:(b + 1) * P, :], in_=hrow[:])

            nc.sync.dma_start(out=stats[:, 0:1], in_=s_sum[:])
            nc.sync.dma_start(out=stats[:, 1:2], in_=s_sq[:])

    return nc


def _emit_scale_shift(nc, constp, psump, statsc, g, be, nf, n_count):
    """Device BN coefficient computation from raw sums.
    statsc [nf,2] (sum, sumsq); returns full [P, nf] scale/shift tiles."""
    mean = constp.tile([nf, 1], F32)
    nc.scalar.mul(mean[:], statsc[:, 0:1], 1.0 / n_count)
    ex2 = constp.tile([nf, 1], F32)
    nc.scalar.mul(ex2[:], statsc[:, 1:2], 1.0 / n_count)
    msq = constp.tile([nf, 1], F32)
    nc.vector.tensor_tensor(out=msq[:], in0=mean[:], in1=mean[:],
                            op=mybir.AluOpType.mult)
    var = constp.tile([nf, 1], F32)
    nc.vector.tensor_tensor(out=var[:], in0=ex2[:], in1=msq[:],
                            op=mybir.AluOpType.subtract)
    veps = constp.tile([nf, 1], F32)
    nc.vector.tensor_scalar_add(veps[:], var[:], BN_EPS)
    std = constp.tile([nf, 1], F32)
    nc.scalar.sqrt(std[:], veps[:])
    istd = constp.tile([nf, 1], F32)
    nc.vector.reciprocal(istd[:], std[:])
    scale_c = constp.tile([nf, 1], F32)
    nc.vector.tensor_tensor(out=scale_c[:], in0=g[:], in1=istd[:],
                            op=mybir.AluOpType.mult)
    mscale = constp.tile([nf, 1], F32)
    nc.vector.tensor_tensor(out=mscale[:], in0=mean[:], in1=scale_c[:],
                            op=mybir.AluOpType.mult)
    shift_c = constp.tile([nf, 1], F32)
    nc.vector.tensor_tensor(out=shift_c[:], in0=be[:], in1=mscale[:],
                            op=mybir.AluOpType.subtract)
    # broadcast to [P, nf] via rank-1 matmul: ones_row^T (x) coef_row
    ident1 = constp.tile([nf, nf], F32)
    make_identity(nc, ident1[:])
    ones_row = constp.tile([1, P], F32)
    nc.vector.memset(ones_row[:], 1.0)
    out_tiles = []
    for i, coef in enumerate((scale_c, shift_c)):
        prow_ps = psump.tile([1, nf], F32, tag=f"prow{i}", space="PSUM")
        nc.tensor.transpose(out=prow_ps[:], in_=coef[:],
                            identity=ident1[0:nf, 0:nf])
        crow = constp.tile([1, nf], F32, tag=f"crow{i}")
        nc.vector.tensor_copy(out=crow[:], in_=prow_ps[:])
        full_ps = psump.tile([P, nf], F32, tag=f"fullp{i}", space="PSUM")
        nc.tensor.matmul(out=full_ps[:], lhsT=ones_row[:], rhs=crow[:],
                         start=True, stop=True)
        full = constp.tile([P, nf], F32, tag=f"coef_full{i}")
        nc.vector.tensor_copy(out=full[:], in_=full_ps[:])
        out_tiles.append(full)
    return out_tiles


def _build_launch2(plan):
    nc = bass.Bass("TRN2", target_bir_lowering=False, debug=False,
                   num_devices=N_CORES)
    tab1 = nc.dram_tensor("tab1", [TABLE_ROWS, H1], F32, kind="ExternalInput")
    stats1 = nc.dram_tensor("stats1", [H1, 2], F32, kind="ExternalInput")
    g1 = nc.dram_tensor("g1", [H1, 1], F32, kind="ExternalInput")
    be1 = nc.dram_tensor("be1", [H1, 1], F32, kind="ExternalInput")
    idx2 = nc.dram_tensor("idx2", [P, plan["TOTC"]], I32, kind="ExternalInput")
    w2a = nc.dram_tensor("w2a", [H1, H2], F32, kind="ExternalInput")
    b2a = nc.dram_tensor("b2a", [H2, 1], F32, kind="ExternalInput")
    w2b = nc.dram_tensor("w2b", [H2, H2], F32, kind="ExternalInput")
    b2b = nc.dram_tensor("b2b", [H2, 1], F32, kind="ExternalInput")
    h2o = nc.dram_tensor("h2o", [SLOTS, H2], F32, kind="ExternalOutput")
    stats = nc.dram_tensor("stats", [H2, 2], F32, kind="ExternalOutput")
    relu_tab = nc.dram_tensor("relu_tab", [TABLE_ROWS + 1, H1], F32,
                              kind="Internal")

    offs, R, starts = plan["offs"], plan["R"], plan["starts"]
    # Phase 1 (own TileContext => hard barrier before phase 2): BN1 coeffs
    # + normalize/relu the full table into relu_tab.
    with tile.TileContext(nc) as tc:
        with (
            tc.tile_pool(name="const1", bufs=1) as constp,
            tc.tile_pool(name="norm", bufs=4) as normp,
            tc.tile_pool(name="psum1", bufs=1, space="PSUM") as psump,
        ):
            stats1_t = constp.tile([H1, 2], F32)
            nc.sync.dma_start(out=stats1_t[:], in_=stats1[:, :])
            g1_t = constp.tile([H1, 1], F32)
            nc.sync.dma_start(out=g1_t[:], in_=g1[:, :])
            be1_t = constp.tile([H1, 1], F32)
            nc.sync.dma_start(out=be1_t[:], in_=be1[:, :])
            scale_full, shift_full = _emit_scale_shift(
                nc, constp, psump, stats1_t, g1_t, be1_t, H1, N_NODES)

            # normalize + relu the full table: 98 strided tiles of 1024 rows
            CH = 8  # rows per partition per tile
            tabv = tab1[:, :].rearrange("(t p j) f -> t p (j f)", p=P, j=CH)
            reluv = relu_tab[0:TABLE_ROWS, :].rearrange("(t p j) f -> t p (j f)", p=P, j=CH)
            n_t = TABLE_ROWS // (P * CH)
            sc_b = scale_full[:].rearrange("p f -> p () f").to_broadcast(
                [P, CH, H1])
            sh_b = shift_full[:].rearrange("p f -> p () f").to_broadcast(
                [P, CH, H1])
            for t in range(n_t):
                nt = normp.tile([P, CH * H1], F32, tag="nt")
                nc.sync.dma_start(out=nt[:], in_=tabv[t])
                sc = normp.tile([P, CH * H1], F32, tag="sc")
                nc.vector.tensor_tensor(
                    out=sc[:].rearrange("p (j f) -> p j f", f=H1),
                    in0=nt[:].rearrange("p (j f) -> p j f", f=H1),
                    in1=sc_b,
                    op=mybir.AluOpType.mult)
                nc.vector.tensor_tensor(
                    out=sc[:].rearrange("p (j f) -> p j f", f=H1),
                    in0=sc[:].rearrange("p (j f) -> p j f", f=H1),
                    in1=sh_b,
                    op=mybir.AluOpType.add)
                nc.scalar.activation(sc[:], sc[:],
                                     mybir.ActivationFunctionType.Relu)
                nc.sync.dma_start(out=reluv[t], in_=sc[:])
            zrow = normp.tile([1, H1], F32, tag="zrow")
            nc.vector.memset(zrow[:], 0.0)
            nc.sync.dma_start(out=relu_tab[TABLE_ROWS:TABLE_ROWS + 1, :],
                              in_=zrow[:])

    # Phase 2: aggregate from relu_tab + MLP2 + stats.
    with tile.TileContext(nc) as tc:
        with (
            tc.tile_pool(name="const", bufs=1) as constp,
            tc.tile_pool(name="idxp", bufs=8) as idxp,
            tc.tile_pool(name="accp", bufs=12) as accp,
            tc.tile_pool(name="work", bufs=6) as workp,
            tc.tile_pool(name="psum", bufs=1, space="PSUM") as psump,
        ):
            ident = constp.tile([P, P], F32)
            make_identity(nc, ident[:])
            # layer-2 weights/consts
            w2a_t = constp.tile([H1, H2], F32)
            nc.sync.dma_start(out=w2a_t[:], in_=w2a[:, :])
            w2b_t = constp.tile([H2, H2], F32)
            nc.sync.dma_start(out=w2b_t[:], in_=w2b[:, :])
            b2a_t = constp.tile([H2, 1], F32)
            nc.sync.dma_start(out=b2a_t[:], in_=b2a[:, :])
            b2b_t = constp.tile([H2, 1], F32)
            nc.sync.dma_start(out=b2b_t[:], in_=b2b[:, :])
            mask = constp.tile([H2, P], F32)
            nc.vector.memset(mask[:], 1.0)
            nc.vector.memset(mask[:, LAST_VALID:], 0.0)
            s_sum = constp.tile([H2, 1], F32)
            s_sq = constp.tile([H2, 1], F32)
            nc.vector.memset(s_sum[:], 0.0)
            nc.vector.memset(s_sq[:], 0.0)

            for b in range(BLOCKS):
                acc = _gather_rounds(nc, tc, (idxp, accp), relu_tab[:, :],
                                     idx2, b, int(starts[b]),
                                     int(R[b]), H1)
                accT = workp.tile([P, P], F32, tag="accT")
                pT = psump.tile([P, P], F32, tag="pT", space="PSUM")
                nc.tensor.transpose(out=pT[:], in_=acc[:], identity=ident[:])
                nc.vector.tensor_copy(out=accT[:], in_=pT[:])
                ps1 = psump.tile([H2, P], F32, tag="ps1", space="PSUM")
                nc.tensor.matmul(out=ps1[:], lhsT=w2a_t[:], rhs=accT[:],
                                 start=True, stop=True)
                r1 = workp.tile([H2, P], F32, tag="r1")
                nc.scalar.activation(r1[:], ps1[:],
                                     mybir.ActivationFunctionType.Relu,
                                     bias=b2a_t[:, 0:1])
                ps2 = psump.tile([H2, P], F32, tag="ps2", space="PSUM")
                nc.tensor.matmul(out=ps2[:], lhsT=w2b_t[:], rhs=r1[:],
                                 start=True, stop=True)
                hpre = workp.tile([H2, P], F32, tag="hpre")
                nc.scalar.activation(hpre[:], ps2[:],
                                     mybir.ActivationFunctionType.Identity,
                                     bias=b2b_t[:, 0:1])
                if b == BLOCKS - 1:
                    hstat = workp.tile([H2, P], F32, tag="hstat")
                    nc.vector.tensor_tensor(out=hstat[:], in0=hpre[:],
                                            in1=mask[:],
                                            op=mybir.AluOpType.mult)
                else:
                    hstat = hpre
                part = workp.tile([H2, 1], F32, tag="part")
                nc.vector.tensor_reduce(out=part[:], in_=hstat[:],
                                        axis=mybir.AxisListType.X,
                                        op=mybir.AluOpType.add)
                nc.vector.tensor_add(out=s_sum[:], in0=s_sum[:], in1=part[:])
                sq = workp.tile([H2, P], F32, tag="sq")
                nc.scalar.square(sq[:], hstat[:])
                part2 = workp.tile([H2, 1], F32, tag="part2")
                nc.vector.tensor_reduce(out=part2[:], in_=sq[:],
                                        axis=mybir.AxisListType.X,
                                        op=mybir.AluOpType.add)
                nc.vector.tensor_add(out=s_sq[:], in0=s_sq[:], in1=part2[:])
                psT = psump.tile([P, H2], F32, tag="psT", space="PSUM")
                nc.tensor.transpose(out=psT[:], in_=hpre[:],
                                    identity=ident[0:H2, 0:H2])
                hrow = workp.tile([P, H2], F32, tag="hrow")
                nc.vector.tensor_copy(out=hrow[:], in_=psT[:])
                nc.sync.dma_start(out=h2o[b * P:(b + 1) * P, :], in_=hrow[:])

            nc.sync.dma_start(out=stats[:, 0:1], in_=s_sum[:])
            nc.sync.dma_start(out=stats[:, 1:2], in_=s_sq[:])

    return nc


def _build_launch3():
    nc = bass.Bass("TRN2", target_bir_lowering=False, debug=False,
                   num_devices=N_CORES)
    h2i = nc.dram_tensor("h2i", [SLOTS, H2], F32, kind="ExternalInput")
    stats2 = nc.dram_tensor("stats2", [H2, 2], F32, kind="ExternalInput")
    g2 = nc.dram_tensor("g2", [H2, 1], F32, kind="ExternalInput")
    be2 = nc.dram_tensor("be2", [H2, 1], F32, kind="ExternalInput")
    outo = nc.dram_tensor("outo", [SLOTS, H2], F32, kind="ExternalOutput")

    with tile.TileContext(nc) as tc:
        with (
            tc.tile_pool(name="const", bufs=1) as constp,
            tc.tile_pool(name="norm", bufs=4) as normp,
            tc.tile_pool(name="psum", bufs=1, space="PSUM") as psump,
        ):
            stats2_t = constp.tile([H2, 2], F32)
            nc.sync.dma_start(out=stats2_t[:], in_=stats2[:, :])
            g2_t = constp.tile([H2, 1], F32)
            nc.sync.dma_start(out=g2_t[:], in_=g2[:, :])
            be2_t = constp.tile([H2, 1], F32)
            nc.sync.dma_start(out=be2_t[:], in_=be2[:, :])
            scale_full, shift_full = _emit_scale_shift(
                nc, constp, psump, stats2_t, g2_t, be2_t, H2, N_NODES)
            CH = 7
            hv = h2i[:, :].rearrange("(t p j) f -> t p (j f)", p=P, j=CH)
            ov = outo[:, :].rearrange("(t p j) f -> t p (j f)", p=P, j=CH)
            n_t = SLOTS // (P * CH)
            sc_b = scale_full[:].rearrange("p f -> p () f").to_broadcast(
                [P, CH, H2])
            sh_b = shift_full[:].rearrange("p f -> p () f").to_broadcast(
                [P, CH, H2])
            for t in range(n_t):
                nt = normp.tile([P, CH * H2], F32, tag="nt")
                nc.sync.dma_start(out=nt[:], in_=hv[t])
                sc = normp.tile([P, CH * H2], F32, tag="sc")
                nc.vector.tensor_tensor(
                    out=sc[:].rearrange("p (j f) -> p j f", f=H2),
                    in0=nt[:].rearrange("p (j f) -> p j f", f=H2),
                    in1=sc_b,
                    op=mybir.AluOpType.mult)
                nc.vector.tensor_tensor(
                    out=sc[:].rearrange("p (j f) -> p j f", f=H2),
                    in0=sc[:].rearrange("p (j f) -> p j f", f=H2),
                    in1=sh_b,
                    op=mybir.AluOpType.add)
                nc.scalar.activation(sc[:], sc[:],
                                     mybir.ActivationFunctionType.Relu)
                nc.sync.dma_start(out=ov[t], in_=sc[:])

    return nc


# ---------------------------------------------------------------- entry
_TRACE = {"enabled": False, "exec_ns": []}


def _run(nc, in_maps):
    _split_sync_waits(nc)
    kw = {}
    if _TRACE["enabled"]:
        kw["trace"] = True
    res = run_bass_kernel_spmd(nc, in_maps, core_ids=list(range(N_CORES)), **kw)
    if _TRACE["enabled"]:
        _TRACE["exec_ns"].append(res.exec_time_ns)
    return res.results


def kernel(**inputs):
    x = np.ascontiguousarray(
        np.vstack([np.asarray(inputs["x"], dtype=np.float32),
                   np.zeros((1, F_IN), np.float32)]))
    plan = _build_plan(np.asarray(inputs["edge_index"]))

    def col(v, n):
        return np.ascontiguousarray(
            np.asarray(v, dtype=np.float32).reshape(n, 1))

    W1a = np.ascontiguousarray(np.asarray(inputs["W1a"], np.float32))
    W1b = np.ascontiguousarray(np.asarray(inputs["W1b"], np.float32))
    W2a = np.ascontiguousarray(np.asarray(inputs["W2a"], np.float32))
    W2b = np.ascontiguousarray(np.asarray(inputs["W2b"], np.float32))

    nc1 = _build_launch1(plan)
    in1 = [
        {"x": x, "idx1": plan["cores"][c]["idx1"], "w1a": W1a,
         "b1a": col(inputs["b1a"], H1), "w1b": W1b,
         "b1b": col(inputs["b1b"], H1)}
        for c in range(N_CORES)
    ]
    res1 = _run(nc1, in1)
    tab1 = np.vstack([res1[c]["h1o"] for c in range(N_CORES)])
    stats1 = np.sum([res1[c]["stats"] for c in range(N_CORES)], axis=0)
    stats1 = np.ascontiguousarray(stats1.astype(np.float32))

    nc2 = _build_launch2(plan)
    in2 = [
        {"tab1": tab1, "stats1": stats1, "g1": col(inputs["g1"], H1),
         "be1": col(inputs["be1"], H1), "idx2": plan["cores"][c]["idx2"],
         "w2a": W2a, "b2a": col(inputs["b2a"], H2), "w2b": W2b,
         "b2b": col(inputs["b2b"], H2)}
        for c in range(N_CORES)
    ]
    res2 = _run(nc2, in2)
    stats2 = np.sum([res2[c]["stats"] for c in range(N_CORES)], axis=0)
    stats2 = np.ascontiguousarray(stats2.astype(np.float32))

    nc3 = _build_launch3()
    in3 = [
        {"h2i": res2[c]["h2o"], "stats2": stats2,
         "g2": col(inputs["g2"], H2), "be2": col(inputs["be2"], H2)}
        for c in range(N_CORES)
    ]
    res3 = _run(nc3, in3)

    out = np.zeros((N_NODES, H2), dtype=np.float32)
    for c in range(N_CORES):
        perm = plan["cores"][c]["perm"]
        out[c * PER_CORE + perm] = res3[c]["outo"][:PER_CORE]
    return out
